# revision 9
# baseline (speedup 1.0000x reference)
"""Trainium2 Bass kernel for nn_BoundaryPredictor2 (ragged_sequence).

Full computation:
  h = l2normalize(hidden); scores = scale * (h@Wq.T) . sum_s(h@Wk.T)
  keep = sigmoid(scores + logistic(noise)) > 0.5
  pooled = kept h tokens packed left, zero padded; loss = binomial log prob of counts.

Algebraic rewrite used on device: sum_s K_s = Wk @ sum_s h_s, and
Q_s . Ksum = h_s . (Wq^T Wk hsum), so scores[s] = h_s . u with
u = (scale * Wq^T Wk) hsum.  No big matmuls remain; the kernel is a
memory-bound two-phase pass per batch row:
  phase A (stream): load hidden tiles, square-accumulate norms (ACT),
    normalize in place (DVE), accumulate hsum (PE ones-matmul).
  phase B: u via small PE matmuls, per-token dot via DVE fused
    multiply-reduce, keep flags + prefix scan -> compaction indices,
    indirect-scatter only the kept rows to the output (which the
    runtime pre-zeroes, so dropped positions stay zero).

Sharding: data-parallel over batch, 2 rows per core on 8 cores.
"""

import math
from contextlib import ExitStack

import numpy as np

B, S, D = 16, 8192, 512
NCORES = 8
RPC = B // NCORES        # batch rows per core
SCALE = (D // 8) ** -0.5
PRIOR = 0.2
PMAJ = 128               # partitions
CPT = S // PMAJ // 8     # 8 column-slices per group... (c per token group)
NSLICE = S // PMAJ       # 64 slices of 128 tokens, token s = p*64 + c
NGROUP = 8               # groups of 8 slices -> [128, 8*512] tiles
BIGIDX = 1000000.0       # scatter sentinel for dropped tokens (OOB, skipped)

_compiled = None


def _build():
    import concourse.bacc as bacc
    import concourse.mybir as mybir
    import concourse.tile as tile
    from concourse.bass import IndirectOffsetOnAxis

    f32 = mybir.dt.float32
    i32 = mybir.dt.int32
    ALU = mybir.AluOpType
    ACT = mybir.ActivationFunctionType

    nc = bacc.Bacc("TRN2", target_bir_lowering=False, debug=False)

    hid_d = nc.dram_tensor("hidden", [RPC, S, D], f32, kind="ExternalInput")
    lgs_d = nc.dram_tensor("logi", [RPC, S], f32, kind="ExternalInput")
    rmat_d = nc.dram_tensor("rmat", [D, D], f32, kind="ExternalInput")
    tri_d = nc.dram_tensor("tri", [PMAJ, PMAJ], f32, kind="ExternalInput")
    pool_d = nc.dram_tensor("pooled", [RPC, S, D], f32, kind="ExternalOutput")
    cnts_d = nc.dram_tensor("cnts", [RPC, PMAJ], f32, kind="ExternalOutput")

    pooled_flat = pool_d.ap().rearrange("r s d -> (r s) d")

    with tile.TileContext(nc) as tc, ExitStack() as ctx:
        from concourse import library_config
        nc.gpsimd.load_library(library_config.standard)
        consts = ctx.enter_context(tc.tile_pool(name="consts", bufs=1))
        hbuf = ctx.enter_context(tc.tile_pool(name="hbuf", bufs=NGROUP))
        sqs = ctx.enter_context(tc.tile_pool(name="sqs", bufs=2))
        prods = ctx.enter_context(tc.tile_pool(name="prods", bufs=2))
        rowp = ctx.enter_context(tc.tile_pool(name="rowp", bufs=2))
        psum = ctx.enter_context(tc.tile_pool(name="psum", bufs=2, space="PSUM"))
        psum1 = ctx.enter_context(tc.tile_pool(name="psum1", bufs=1, space="PSUM"))

        # constants
        rmat_s = consts.tile([PMAJ, 4, D], f32)
        nc.sync.dma_start(rmat_s[:], rmat_d.ap().rearrange("(k e) j -> e k j", e=PMAJ))
        tri_s = consts.tile([PMAJ, PMAJ], f32)
        nc.sync.dma_start(tri_s[:], tri_d.ap())
        ones_col = consts.tile([PMAJ, 1], f32)
        nc.vector.memset(ones_col[:], 1.0)
        ones11 = consts.tile([1, 1], f32)
        nc.vector.memset(ones11[:], 1.0)
        ones_row = consts.tile([1, PMAJ], f32)
        nc.vector.memset(ones_row[:], 1.0)
        zeros64 = consts.tile([PMAJ, NSLICE], f32)
        nc.vector.memset(zeros64[:], 0.0)
        big64 = consts.tile([PMAJ, NSLICE], f32)
        nc.vector.memset(big64[:], BIGIDX)

        for r in range(RPC):
            hid_r = hid_d.ap()[r].rearrange("(p c) d -> p c d", p=PMAJ)  # [128,64,512]
            ss = rowp.tile([PMAJ, NSLICE], f32, tag="ss")
            rt = rowp.tile([PMAJ, NSLICE], f32, tag="rt")
            rs = rowp.tile([PMAJ, NSLICE], f32, tag="rs")
            dots = rowp.tile([PMAJ, NSLICE], f32, tag="dots")
            hsum_ps = psum.tile([1, D], f32, tag="hsum_ps")

            hg = []
            for g in range(NGROUP):
                hG = hbuf.tile([PMAJ, 8, D], f32, tag="hbuf")
                nc.sync.dma_start(hG[:], hid_r[:, 8 * g:8 * g + 8, :])
                hg.append(hG)
                for cl in range(8):
                    c = 8 * g + cl
                    sl = hG[:, cl, :]
                    sq = sqs.tile([PMAJ, D], f32, tag="sq")
                    nc.scalar.activation(sq[:], sl, ACT.Square,
                                         accum_out=ss[:, c:c + 1])
                # batched rsqrt for the group's 8 columns
                gsl = slice(8 * g, 8 * g + 8)
                nc.scalar.activation(rt[:, gsl], ss[:, gsl], ACT.Sqrt)
                nc.vector.reciprocal(rs[:, gsl], rt[:, gsl])
                for cl in range(8):
                    c = 8 * g + cl
                    sl = hG[:, cl, :]
                    nc.vector.tensor_scalar_mul(sl, sl, rs[:, c:c + 1])
                    nc.tensor.matmul(hsum_ps[:], ones_col[:], sl,
                                     start=(c == 0), stop=(c == NSLICE - 1))

            # ---- phase B: u = rmat^T-style matvec, dots, indices, scatter
            hsum_sb = rowp.tile([1, D], f32, tag="hsum_sb")
            nc.vector.tensor_copy(hsum_sb[:], hsum_ps[:])
            hsumT = rowp.tile([PMAJ, 4], f32, tag="hsumT")
            for k in range(4):
                col_ps = psum1.tile([PMAJ, 1], f32, tag="col_ps")
                nc.tensor.matmul(col_ps[:],
                                 hsum_sb[:, 128 * k:128 * k + 128], ones11[:],
                                 start=True, stop=True)
                nc.vector.tensor_copy(hsumT[:, k:k + 1], col_ps[:])
            u_ps = psum1.tile([1, D], f32, tag="u_ps")
            for k in range(4):
                nc.tensor.matmul(u_ps[:], hsumT[:, k:k + 1], rmat_s[:, k, :],
                                 start=(k == 0), stop=(k == 3))
            u_sb = rowp.tile([1, D], f32, tag="u_sb")
            nc.vector.tensor_copy(u_sb[:], u_ps[:])
            u_bc = rowp.tile([PMAJ, D], f32, tag="u_bc")
            nc.gpsimd.partition_broadcast(u_bc[:], u_sb[:])

            for c in range(NSLICE):
                g, cl = divmod(c, 8)
                pr = prods.tile([PMAJ, D], f32, tag="pr")
                nc.vector.tensor_tensor(pr[:], hg[g][:, cl, :], u_bc[:],
                                        op=ALU.mult)
                nc.vector.tensor_reduce(dots[:, c:c + 1], pr[:],
                                        axis=mybir.AxisListType.X, op=ALU.add)

            lg_t = rowp.tile([PMAJ, NSLICE], f32, tag="lg")
            nc.sync.dma_start(lg_t[:], lgs_d.ap()[r].rearrange("(p c) -> p c", p=PMAJ))
            xx = rowp.tile([PMAJ, NSLICE], f32, tag="xx")
            nc.vector.tensor_add(xx[:], dots[:], lg_t[:])
            keep = rowp.tile([PMAJ, NSLICE], f32, tag="keep")
            nc.vector.tensor_scalar(keep[:], xx[:], 0.0, None, op0=ALU.is_gt)
            incl = rowp.tile([PMAJ, NSLICE], f32, tag="incl")
            nc.vector.tensor_tensor_scan(incl[:], keep[:], zeros64[:], 0.0,
                                         op0=ALU.add, op1=ALU.add)
            nc.sync.dma_start(cnts_d.ap()[r].rearrange("(p o) -> p o", o=1),
                              incl[:, NSLICE - 1:NSLICE])
            excl_ps = psum1.tile([PMAJ, 1], f32, tag="excl_ps")
            nc.tensor.matmul(excl_ps[:], tri_s[:], incl[:, NSLICE - 1:NSLICE],
                             start=True, stop=True)
            excl1 = rowp.tile([PMAJ, 1], f32, tag="excl1")
            nc.vector.tensor_scalar_add(excl1[:], excl_ps[:], float(r * S - 1))
            basef = rowp.tile([PMAJ, NSLICE], f32, tag="basef")
            nc.vector.tensor_scalar_add(basef[:], incl[:], excl1[:])
            keep_u8 = rowp.tile([PMAJ, NSLICE], mybir.dt.uint8, tag="keepu8")
            nc.vector.tensor_copy(keep_u8[:], keep[:])
            destf = rowp.tile([PMAJ, NSLICE], f32, tag="destf")
            nc.vector.tensor_copy(destf[:], big64[:])
            nc.vector.copy_predicated(destf[:], keep_u8[:], basef[:])
            desti = rowp.tile([PMAJ, NSLICE], i32, tag="desti")
            nc.vector.tensor_copy(desti[:], destf[:])

            for c in range(NSLICE):
                g, cl = divmod(c, 8)
                nc.gpsimd.indirect_dma_start(
                    out=pooled_flat,
                    out_offset=IndirectOffsetOnAxis(
                        ap=desti[:, c:c + 1], axis=0),
                    in_=hg[g][:, cl, :],
                    in_offset=None,
                    bounds_check=RPC * S - 1,
                    oob_is_err=False,
                )

    nc.compile()
    return nc


def _get_compiled():
    global _compiled
    if _compiled is None:
        _compiled = _build()
    return _compiled


def kernel(hidden: np.ndarray, Wq: np.ndarray, Wk: np.ndarray,
           noise: np.ndarray):
    from concourse import bass_utils

    nc = _get_compiled()

    hidden = np.ascontiguousarray(hidden, dtype=np.float32)
    n64 = noise.astype(np.float64)
    logi = (np.log(n64) - np.log1p(-n64)).astype(np.float32)
    rmat = ((Wk.astype(np.float64).T @ Wq.astype(np.float64)) * SCALE).astype(np.float32)
    tri = np.triu(np.ones((PMAJ, PMAJ), np.float32), 1)  # tri[q,p]=1 iff q<p

    in_maps = [
        {
            "hidden": hidden[RPC * i:RPC * (i + 1)],
            "logi": logi[RPC * i:RPC * (i + 1)],
            "rmat": rmat,
            "tri": tri,
        }
        for i in range(NCORES)
    ]
    res = bass_utils.run_bass_kernel_spmd(nc, in_maps, core_ids=list(range(NCORES)))

    pooled = np.concatenate([r["pooled"] for r in res.results], axis=0)
    counts = np.concatenate([r["cnts"].sum(axis=1) for r in res.results], axis=0)

    n = float(S)
    lp = np.array([
        math.lgamma(n + 1.0) - math.lgamma(k + 1.0) - math.lgamma(n - k + 1.0)
        + k * math.log(PRIOR) + (n - k) * math.log1p(-PRIOR)
        for k in counts.astype(np.float64)
    ])
    loss = np.float32(-(lp.mean()) / n)
    return pooled, loss


# revision 18
# speedup vs baseline: 1.5863x; 1.5863x over previous
"""Trainium2 Bass kernel for nn_BoundaryPredictor2 (ragged_sequence).

Full computation:
  h = l2normalize(hidden); scores = scale * (h@Wq.T) . sum_s(h@Wk.T)
  keep = sigmoid(scores + logistic(noise)) > 0.5
  pooled = kept h tokens packed left, zero padded; loss = binomial log prob of counts.

Algebraic rewrite used on device: sum_s K_s = Wk @ sum_s h_s, and
Q_s . Ksum = h_s . (Wq^T Wk hsum), so scores[s] = h_s . u with
u = (scale * Wq^T Wk) hsum.  No big matmuls remain; the kernel is a
memory-bound two-phase pass per batch row:
  phase A (stream): load hidden tiles, square-accumulate norms (ACT),
    normalize in place (DVE), accumulate hsum (PE ones-matmul).
  phase B: u via small PE matmuls, per-token dot via DVE fused
    multiply-reduce, keep flags + prefix scan -> compaction indices,
    indirect-scatter only the kept rows to the output (which the
    runtime pre-zeroes, so dropped positions stay zero).

Sharding: data-parallel over batch, 2 rows per core on 8 cores.
"""

import math
from contextlib import ExitStack

import numpy as np

B, S, D = 16, 8192, 512
NCORES = 8
RPC = B // NCORES        # batch rows per core
SCALE = (D // 8) ** -0.5
PRIOR = 0.2
PMAJ = 128               # partitions
CPT = S // PMAJ // 8     # 8 column-slices per group... (c per token group)
NSLICE = S // PMAJ       # 64 slices of 128 tokens, token s = p*64 + c
NGROUP = 8               # groups of 8 slices -> [128, 8*512] tiles
BIGIDX = 1000000.0       # scatter sentinel for dropped tokens (OOB, skipped)

_compiled = None

# Build stages for differential profiling (bench3.py):
# 0 = I/O decl + const loads only; 1 = + hidden loads; 2 = + phase A
# (squares/normalize/hsum); 3 = + phase B scores/indices; 4 = full (scatter).
STAGE = 4


def _build():
    import concourse.bacc as bacc
    import concourse.mybir as mybir
    import concourse.tile as tile
    from concourse.bass import IndirectOffsetOnAxis

    f32 = mybir.dt.float32
    i32 = mybir.dt.int32
    ALU = mybir.AluOpType
    ACT = mybir.ActivationFunctionType

    nc = bacc.Bacc("TRN2", target_bir_lowering=False, debug=False)

    hid_d = nc.dram_tensor("hidden", [RPC, S, D], f32, kind="ExternalInput")
    lgs_d = nc.dram_tensor("logi", [RPC, S], f32, kind="ExternalInput")
    rmat_d = nc.dram_tensor("rmat", [D, D], f32, kind="ExternalInput")
    tri_d = nc.dram_tensor("tri", [PMAJ, PMAJ], f32, kind="ExternalInput")
    pool_d = nc.dram_tensor("pooled", [RPC, S, D], f32, kind="ExternalOutput")
    cnts_d = nc.dram_tensor("cnts", [RPC, PMAJ], f32, kind="ExternalOutput")

    pooled_flat = pool_d.ap().rearrange("r s d -> (r s) d")

    with tile.TileContext(nc) as tc, ExitStack() as ctx:
        from concourse import library_config
        nc.gpsimd.load_library(library_config.standard)
        consts = ctx.enter_context(tc.tile_pool(name="consts", bufs=1))
        hbuf = ctx.enter_context(tc.tile_pool(name="hbuf", bufs=NGROUP))
        sqs = ctx.enter_context(tc.tile_pool(name="sqs", bufs=2))
        prods = ctx.enter_context(tc.tile_pool(name="prods", bufs=2))
        rowp = ctx.enter_context(tc.tile_pool(name="rowp", bufs=2))
        psum = ctx.enter_context(tc.tile_pool(name="psum", bufs=2, space="PSUM"))
        psum1 = ctx.enter_context(tc.tile_pool(name="psum1", bufs=1, space="PSUM"))

        # constants
        rmat_s = consts.tile([PMAJ, 4, D], f32)
        nc.sync.dma_start(rmat_s[:], rmat_d.ap().rearrange("(k e) j -> e k j", e=PMAJ))
        tri_s = consts.tile([PMAJ, PMAJ], f32)
        nc.sync.dma_start(tri_s[:], tri_d.ap())
        ones_col = consts.tile([PMAJ, 1], f32)
        nc.vector.memset(ones_col[:], 1.0)
        ones11 = consts.tile([1, 1], f32)
        nc.vector.memset(ones11[:], 1.0)
        ones_row = consts.tile([1, PMAJ], f32)
        nc.vector.memset(ones_row[:], 1.0)
        zeros64 = consts.tile([PMAJ, NSLICE], f32)
        nc.vector.memset(zeros64[:], 0.0)
        big64 = consts.tile([PMAJ, NSLICE], f32)
        nc.vector.memset(big64[:], BIGIDX)

        for r in range(RPC):
            hid_r = hid_d.ap()[r].rearrange("(p c) d -> p c d", p=PMAJ)  # [128,64,512]
            ss = rowp.tile([PMAJ, NSLICE], f32, tag="ss")
            rt = rowp.tile([PMAJ, NSLICE], f32, tag="rt")
            rs = rowp.tile([PMAJ, NSLICE], f32, tag="rs")
            dots = rowp.tile([PMAJ, NSLICE], f32, tag="dots")
            hsum_ps = psum.tile([1, D], f32, tag="hsum_ps")

            hg = []
            norm_bis = [None] * NSLICE
            for g in range(NGROUP):
                hG = hbuf.tile([PMAJ, 8, D], f32, tag="hbuf")
                if STAGE >= 1:
                    nc.sync.dma_start(hG[:], hid_r[:, 8 * g:8 * g + 8, :])
                hg.append(hG)
                if STAGE < 2:
                    continue
                for cl in range(8):
                    c = 8 * g + cl
                    sl = hG[:, cl, :]
                    sq = sqs.tile([PMAJ, D], f32, tag="sq")
                    nc.scalar.activation(sq[:], sl, ACT.Square,
                                         accum_out=ss[:, c:c + 1])
                # batched rsqrt for the group's 8 columns
                gsl = slice(8 * g, 8 * g + 8)
                nc.scalar.activation(rt[:, gsl], ss[:, gsl], ACT.Sqrt)
                nc.vector.reciprocal(rs[:, gsl], rt[:, gsl])
                for cl in range(8):
                    c = 8 * g + cl
                    sl = hG[:, cl, :]
                    norm_bis[c] = nc.vector.tensor_scalar_mul(sl, sl, rs[:, c:c + 1])
                    nc.tensor.matmul(hsum_ps[:], ones_col[:], sl,
                                     start=(c == 0), stop=(c == NSLICE - 1))
            if STAGE < 3:
                continue

            # ---- phase B: u = rmat^T-style matvec, dots, indices, scatter
            hsum_sb = rowp.tile([1, D], f32, tag="hsum_sb")
            nc.vector.tensor_copy(hsum_sb[:], hsum_ps[:])
            hsumT = rowp.tile([PMAJ, 4], f32, tag="hsumT")
            for k in range(4):
                col_ps = psum1.tile([PMAJ, 1], f32, tag="col_ps")
                nc.tensor.matmul(col_ps[:],
                                 hsum_sb[:, 128 * k:128 * k + 128], ones11[:],
                                 start=True, stop=True)
                nc.vector.tensor_copy(hsumT[:, k:k + 1], col_ps[:])
            u_ps = psum1.tile([1, D], f32, tag="u_ps")
            for k in range(4):
                nc.tensor.matmul(u_ps[:], hsumT[:, k:k + 1], rmat_s[:, k, :],
                                 start=(k == 0), stop=(k == 3))
            u_sb = rowp.tile([1, D], f32, tag="u_sb")
            nc.vector.tensor_copy(u_sb[:], u_ps[:])
            u_bc = rowp.tile([PMAJ, D], f32, tag="u_bc")
            nc.gpsimd.partition_broadcast(u_bc[:], u_sb[:])

            for c in range(NSLICE):
                g, cl = divmod(c, 8)
                pr = prods.tile([PMAJ, D], f32, tag="pr")
                nc.vector.scalar_tensor_tensor(
                    out=pr[:], in0=hg[g][:, cl, :], scalar=0.0, in1=u_bc[:],
                    op0=ALU.add, op1=ALU.mult, accum_out=dots[:, c:c + 1])

            lg_t = rowp.tile([PMAJ, NSLICE], f32, tag="lg")
            nc.sync.dma_start(lg_t[:], lgs_d.ap()[r].rearrange("(p c) -> p c", p=PMAJ))
            xx = rowp.tile([PMAJ, NSLICE], f32, tag="xx")
            nc.vector.tensor_add(xx[:], dots[:], lg_t[:])
            keep = rowp.tile([PMAJ, NSLICE], f32, tag="keep")
            nc.vector.tensor_scalar(keep[:], xx[:], 0.0, None, op0=ALU.is_gt)
            incl = rowp.tile([PMAJ, NSLICE], f32, tag="incl")
            nc.vector.tensor_tensor_scan(incl[:], keep[:], zeros64[:], 0.0,
                                         op0=ALU.add, op1=ALU.add)
            nc.sync.dma_start(cnts_d.ap()[r].rearrange("(p o) -> p o", o=1),
                              incl[:, NSLICE - 1:NSLICE])
            excl_ps = psum1.tile([PMAJ, 1], f32, tag="excl_ps")
            nc.tensor.matmul(excl_ps[:], tri_s[:], incl[:, NSLICE - 1:NSLICE],
                             start=True, stop=True)
            excl1 = rowp.tile([PMAJ, 1], f32, tag="excl1")
            nc.vector.tensor_scalar_add(excl1[:], excl_ps[:], float(r * S - 1))
            basef = rowp.tile([PMAJ, NSLICE], f32, tag="basef")
            nc.vector.tensor_scalar_add(basef[:], incl[:], excl1[:])
            keep_u8 = rowp.tile([PMAJ, NSLICE], mybir.dt.uint8, tag="keepu8")
            nc.vector.tensor_copy(keep_u8[:], keep[:])
            destf = rowp.tile([PMAJ, NSLICE], f32, tag="destf")
            nc.vector.tensor_copy(destf[:], big64[:])
            nc.vector.copy_predicated(destf[:], keep_u8[:], basef[:])
            desti = rowp.tile([PMAJ, NSLICE], i32, tag="desti")
            cast_bi = nc.vector.tensor_copy(desti[:], destf[:])

            if STAGE < 4:
                continue
            # The 64 indirect scatters write provably-disjoint rows of
            # `pooled` (dest indices are unique; OOB sentinels skipped), but
            # Tile's tracker sees whole-tensor WAW and would chain each call
            # on the previous one's completion (~2.6us each).  Emit them
            # dep-free via dep-state snapshot/restore; order is then enforced
            # manually: an explicit semaphore barrier on Pool, plus visible
            # touch-writes on the hG tiles so the next row's loads WAR-wait
            # until the scatter reads have drained.
            from concourse.tile import add_dep_helper
            gsems = [nc.alloc_semaphore(f"scat{r}_{g}") for g in range(NGROUP)]
            scat_bis = []
            for c in range(NSLICE):
                g, cl = divmod(c, 8)
                if STAGE == 5:
                    # model-only proxy: same payload, static destination
                    nc.gpsimd.dma_start(
                        pooled_flat[128 * (64 * r + c):128 * (64 * r + c) + 128, :],
                        hg[g][:, cl, :])
                else:
                    st = tc.get_tile_state()
                    bi = nc.gpsimd.indirect_dma_start(
                        out=pooled_flat,
                        out_offset=IndirectOffsetOnAxis(
                            ap=desti[:, c:c + 1], axis=0),
                        in_=hg[g][:, cl, :],
                        in_offset=None,
                        bounds_check=RPC * S - 1,
                        oob_is_err=False,
                    ).then_inc(gsems[g], 16)
                    tc.load_tile_state(st)
                    add_dep_helper(bi.ins, cast_bi.ins,
                                   reason="scatter reads desti")
                    add_dep_helper(bi.ins, norm_bis[c].ins,
                                   reason="scatter reads normalized hG slice")
                    scat_bis.append(bi)
            if STAGE != 5:
                # all 64 issued first; then per-group completion waits so the
                # next row's load of slot g unblocks as soon as group g drains
                last = scat_bis[-1]
                for g in range(NGROUP):
                    w_bi = nc.gpsimd.wait_ge(gsems[g], 8 * 16)
                    add_dep_helper(w_bi.ins, last.ins, sync=False,
                                   reason="waits after all scatter issues")
                    m_bi = nc.gpsimd.memset(hg[g][:, 0, 0:1], 0.0)
                    add_dep_helper(m_bi.ins, w_bi.ins, sync=False,
                                   reason="slot release after group drain")

    nc.compile()
    return nc


def _get_compiled():
    global _compiled
    if _compiled is None:
        _compiled = _build()
    return _compiled


def kernel(hidden: np.ndarray, Wq: np.ndarray, Wk: np.ndarray,
           noise: np.ndarray):
    from concourse import bass_utils

    nc = _get_compiled()

    hidden = np.ascontiguousarray(hidden, dtype=np.float32)
    n64 = noise.astype(np.float64)
    logi = (np.log(n64) - np.log1p(-n64)).astype(np.float32)
    rmat = ((Wk.astype(np.float64).T @ Wq.astype(np.float64)) * SCALE).astype(np.float32)
    tri = np.triu(np.ones((PMAJ, PMAJ), np.float32), 1)  # tri[q,p]=1 iff q<p

    in_maps = [
        {
            "hidden": hidden[RPC * i:RPC * (i + 1)],
            "logi": logi[RPC * i:RPC * (i + 1)],
            "rmat": rmat,
            "tri": tri,
        }
        for i in range(NCORES)
    ]
    res = bass_utils.run_bass_kernel_spmd(nc, in_maps, core_ids=list(range(NCORES)))

    pooled = np.concatenate([r["pooled"] for r in res.results], axis=0)
    counts = np.concatenate([r["cnts"].sum(axis=1) for r in res.results], axis=0)

    n = float(S)
    lp = np.array([
        math.lgamma(n + 1.0) - math.lgamma(k + 1.0) - math.lgamma(n - k + 1.0)
        + k * math.log(PRIOR) + (n - k) * math.log1p(-PRIOR)
        for k in counts.astype(np.float64)
    ])
    loss = np.float32(-(lp.mean()) / n)
    return pooled, loss
